# revision 12
# baseline (speedup 1.0000x reference)
"""2-layer GCN on 8 Trainium2 NeuronCores (Bass/Tile, SPMD).

Per core (core k owns dst nodes [12500k, 12500(k+1))):
  A) table1 rows [12544k, 12544(k+1)) = dinv * (x @ W1) in bf16 (sharded),
     then AllGather -> full table1 (row == node id within shard blocks,
     padded to 100352 rows).
  B) L1 aggregation: per dst-window (128 nodes) and src-segment (32768 rows,
     int16 dma_gather limit), bulk-gather table1[src] then scatter-add via
     selection matmuls (Sel[e,d] = (dstoff[e]==d)) accumulating in PSUM.
     h1 = relu(dinv * sum + b1) -> DRAM.
  C) table2 shard = dinv * (h1 @ W2) (via dma-transposed h1), AllGather ->
     full table2 with row == node id.
  D) L2 aggregation (same schedule/indices as B): out = dinv * sum + b2.
Host: concat shards.

Note: table1 after stage A has node n at row pad_row(n) = 12544*(n//12544)
+ ... -- rows are just n (cores compute contiguous 12544-row blocks), so
row == node id; rows >= 100000 are junk and never gathered.

Numerics: tables bf16, PSUM accumulate fp32, epilogues fp32.
"""

import numpy as np

N = 100000
F = 128
HID = 64
CLS = 40
CORES = 8
SHARD = 12500
P = 128
WINS = 98  # ceil(12544/128)
SHARD_PAD = WINS * P  # 12544
SEG = 32768
NSEG = 4
ELEM1 = 128  # bf16 elems per table row = 256B (real payload in first cols)
T1_ROWS = SHARD_PAD * CORES  # 100352


def _bf16(a):
    import ml_dtypes

    return np.asarray(a, dtype=ml_dtypes.bfloat16)


def _preprocess(edge_index):
    src = np.concatenate([edge_index[0].astype(np.int64), np.arange(N, dtype=np.int64)])
    dst = np.concatenate([edge_index[1].astype(np.int64), np.arange(N, dtype=np.int64)])
    deg = np.bincount(dst, minlength=N).astype(np.float64)
    dinv = np.where(deg > 0, 1.0 / np.sqrt(deg), 0.0).astype(np.float32)

    per_core = []
    counts = np.zeros((CORES, WINS, NSEG), dtype=np.int64)
    for k in range(CORES):
        m = (dst >= SHARD * k) & (dst < SHARD * (k + 1))
        s_k = src[m]
        local = dst[m] - SHARD * k
        win = local // P
        off = local % P
        seg = s_k // SEG
        order = np.lexsort((seg, win))
        s_k, win, off, seg = s_k[order], win[order], off[order], seg[order]
        np.add.at(counts[k], (win, seg), 1)
        per_core.append((s_k, win, off, seg))

    gmax = counts.max(axis=0)  # [WINS, NSEG]
    gm16 = ((gmax + 15) // 16) * 16  # idx counts per (w,s) call, common

    calls = []  # (w, s, idx_off16, num_idxs, n_groups, tile_base)
    idx_off = 0
    n_tiles = 0
    for w in range(WINS):
        for s in range(NSEG):
            ni = int(gm16[w, s])
            if ni == 0:
                continue
            ng = (ni + P - 1) // P
            assert ni <= 1024, f"gather call too large: {ni}"
            calls.append((w, s, idx_off // 16, ni, ng, n_tiles))
            idx_off += ni
            n_tiles += ng
    n_idx = idx_off
    assert n_idx % 16 == 0

    first_tile = {}
    last_tile = {}
    t = 0
    for w, s, _, _, ng, tb in calls:
        for g in range(ng):
            if w not in first_tile:
                first_tile[w] = t
            last_tile[w] = t
            t += 1

    data = []
    for k in range(CORES):
        s_k, win, off, seg = per_core[k]
        idx16 = np.zeros(n_idx, dtype=np.int16)
        dstoff = np.full(n_tiles * P, -1.0, dtype=np.float32)
        key = win * NSEG + seg
        starts = np.concatenate(([0], np.flatnonzero(np.diff(key)) + 1))
        ends = np.concatenate((starts[1:], [len(key)]))
        bounds = {int(key[st]): (int(st), int(en)) for st, en in zip(starts, ends)}
        for w, s, off16, ni, ng, tb in calls:
            i, j = bounds.get(w * NSEG + s, (0, 0))
            n_e = j - i
            st = off16 * 16
            if n_e > 0:
                idx16[st : st + n_e] = (s_k[i:j] - SEG * s).astype(np.int16)
                dstoff[tb * P : tb * P + n_e] = off[i:j].astype(np.float32)
        data.append((idx16, dstoff))

    sched = {
        "calls": calls,
        "n_idx": n_idx,
        "n_tiles": n_tiles,
        "first_tile": first_tile,
        "last_tile": last_tile,
    }
    return sched, data, dinv


class _EndStagesExc(Exception):
    pass


def _build_program(sched, stages="ABCD", repeat=1):
    import concourse.mybir as mybir
    import concourse.tile as tile
    from concourse import bacc
    from concourse import library_config

    calls = sched["calls"]
    n_idx = sched["n_idx"]
    n_tiles = sched["n_tiles"]
    first_tile = sched["first_tile"]
    last_tile = sched["last_tile"]
    max_ng = max(c[4] for c in calls)

    nc = bacc.Bacc("TRN2", target_bir_lowering=False, debug=False, num_devices=CORES, num_swdge_queues=4)
    dt = mybir.dt

    xT = nc.dram_tensor("xT", [P, SHARD_PAD], dt.bfloat16, kind="ExternalInput")
    w1 = nc.dram_tensor("w1", [F, HID], dt.bfloat16, kind="ExternalInput")
    w2 = nc.dram_tensor("w2", [HID, CLS], dt.bfloat16, kind="ExternalInput")
    b1r = nc.dram_tensor("b1r", [P, HID], dt.float32, kind="ExternalInput")
    b2r = nc.dram_tensor("b2r", [P, CLS], dt.float32, kind="ExternalInput")
    dinv_ch = nc.dram_tensor("dinv_ch", [P, WINS], dt.float32, kind="ExternalInput")
    dinv_wn = nc.dram_tensor("dinv_wn", [P, WINS], dt.float32, kind="ExternalInput")
    iota_in = nc.dram_tensor("iota", [P, P], dt.float32, kind="ExternalInput")
    idx1_in = nc.dram_tensor("idx1", [16, n_idx // 16], dt.int16, kind="ExternalInput")
    dstoff_in = nc.dram_tensor("dstoff", [P, n_tiles], dt.float32, kind="ExternalInput")
    out_ext = nc.dram_tensor("out", [SHARD_PAD, CLS], dt.float32, kind="ExternalOutput")

    table1 = nc.dram_tensor(
        "table1", [T1_ROWS, ELEM1], dt.bfloat16, addr_space="Shared"
    )
    ag1_in = nc.dram_tensor("ag1_in", [SHARD_PAD, ELEM1], dt.bfloat16)
    h1_dram = nc.dram_tensor("h1_dram", [SHARD_PAD, HID], dt.bfloat16)
    ag_in = nc.dram_tensor("ag_in", [SHARD, ELEM1], dt.bfloat16)
    table2 = nc.dram_tensor("table2", [N, ELEM1], dt.bfloat16, addr_space="Shared")

    with tile.TileContext(nc) as tc:
        with (
            tc.tile_pool(name="const", bufs=1) as const,
            tc.tile_pool(name="xs", bufs=3) as xs,
            tc.tile_pool(name="apsum", bufs=2, space="PSUM") as apsum,
            tc.tile_pool(name="aout", bufs=3) as aout,
            tc.tile_pool(name="msg", bufs=8) as msgp,
            tc.tile_pool(name="sel", bufs=6) as selp,
            tc.tile_pool(name="wpsum", bufs=2, space="PSUM") as wpsum,
            tc.tile_pool(name="epi", bufs=6) as epi,
        ):
          try:
            nc.gpsimd.load_library(library_config.mlp)

            w1_t = const.tile([F, HID], dt.bfloat16)
            nc.sync.dma_start(out=w1_t[:], in_=w1[:, :])
            w2_t = const.tile([HID, CLS], dt.bfloat16)
            nc.sync.dma_start(out=w2_t[:], in_=w2[:, :])
            b1_t = const.tile([P, HID], dt.float32)
            nc.sync.dma_start(out=b1_t[:], in_=b1r[:, :])
            b2_t = const.tile([P, CLS], dt.float32)
            nc.sync.dma_start(out=b2_t[:], in_=b2r[:, :])
            dinv_ch_t = const.tile([P, WINS], dt.float32)
            nc.sync.dma_start(out=dinv_ch_t[:], in_=dinv_ch[:, :])
            dinv_wn_t = const.tile([P, WINS], dt.float32)
            nc.sync.dma_start(out=dinv_wn_t[:], in_=dinv_wn[:, :])
            iota_t = const.tile([P, P], dt.float32)
            nc.sync.dma_start(out=iota_t[:], in_=iota_in[:, :])
            idx1_t = const.tile([P, n_idx // 16], dt.int16)
            for r in range(8):
                nc.sync.dma_start(
                    out=idx1_t[16 * r : 16 * (r + 1), :], in_=idx1_in[:, :]
                )
            dstoff_t = const.tile([P, n_tiles], dt.float32)
            nc.sync.dma_start(out=dstoff_t[:], in_=dstoff_in[:, :])

            for _rep in range(repeat):
                # ---- Stage A: own table1 rows = dinv * (x @ W1); AllGather
                GB = 4
                for cb in range(0, WINS, GB):
                    nb = min(GB, WINS - cb)
                    xt = xs.tile([P, GB * P], dt.bfloat16, tag="xt")
                    nc.sync.dma_start(
                        out=xt[:, : nb * P], in_=xT[:, cb * P : (cb + nb) * P]
                    )
                    ps = apsum.tile([P, GB * HID], dt.float32, space="PSUM", tag="aps")
                    ob = aout.tile([P, GB * ELEM1], dt.bfloat16, tag="ob")
                    for j in range(nb):
                        nc.tensor.matmul(
                            out=ps[:, j * HID : (j + 1) * HID],
                            lhsT=xt[:, j * P : (j + 1) * P],
                            rhs=w1_t[:, :],
                            start=True,
                            stop=True,
                        )
                        if ((cb // GB) + j) % 2 == 0:
                            nc.scalar.activation(
                                ob[:, j * ELEM1 : j * ELEM1 + HID],
                                ps[:, j * HID : (j + 1) * HID],
                                mybir.ActivationFunctionType.Copy,
                                scale=dinv_ch_t[:, cb + j : cb + j + 1],
                            )
                        else:
                            nc.vector.tensor_scalar_mul(
                                ob[:, j * ELEM1 : j * ELEM1 + HID],
                                ps[:, j * HID : (j + 1) * HID],
                                dinv_ch_t[:, cb + j : cb + j + 1],
                            )
                    nc.sync.dma_start(
                        out=ag1_in[:, :].rearrange("(c p) e -> p c e", p=P)[
                            :, cb : cb + nb, :
                        ],
                        in_=ob[:, : nb * ELEM1].rearrange("p (c e) -> p c e", e=ELEM1),
                    )

                if "A2" not in stages:
                    nc.gpsimd.collective_compute(
                        "AllGather",
                        mybir.AluOpType.bypass,
                        replica_groups=[list(range(CORES))],
                        ins=[ag1_in.ap().opt()],
                        outs=[table1.ap().opt()],
                    )

                # ---- shared aggregation layer
                def agg_layer(table_ap, table_rows, d_out, out_cb):
                    cur_psum = [None]
                    for ci, (w, s, off16, ni, ng, tb) in enumerate(calls):
                        mt = msgp.tile([P, max_ng * ELEM1], dt.bfloat16, tag="mt")
                        if ci < 8 and _rep == 0:
                            nc.vector.memset(mt[:], 0.0)
                        nc.gpsimd.dma_gather(
                            out_ap=mt[:, : ng * ELEM1].rearrange(
                                "p (g e) -> p g e", e=ELEM1
                            ),
                            in_ap=table_ap[SEG * s : min(SEG * (s + 1), table_rows), :],
                            idxs_ap=idx1_t[:, off16 : off16 + ni // 16],
                            num_idxs=ni,
                            num_idxs_reg=ni,
                            elem_size=ELEM1,
                            queue_num=ci % 4,
                        )
                        for g in range(ng):
                            t = tb + g
                            if t == first_tile[w]:
                                wps = wpsum.tile(
                                    [P, d_out], dt.float32, space="PSUM", tag="wps"
                                )
                                cur_psum[0] = wps
                            sel = selp.tile([P, P], dt.bfloat16, tag="sel")
                            nc.vector.tensor_scalar(
                                sel[:],
                                iota_t[:],
                                dstoff_t[:, t : t + 1],
                                None,
                                mybir.AluOpType.is_equal,
                            )
                            nc.tensor.matmul(
                                out=cur_psum[0][:],
                                lhsT=sel[:],
                                rhs=mt[:, g * ELEM1 : g * ELEM1 + d_out],
                                start=(t == first_tile[w]),
                                stop=(t == last_tile[w]),
                            )
                            if t == last_tile[w]:
                                t1 = epi.tile([P, d_out], dt.float32, tag="t1")
                                nc.scalar.activation(
                                    t1[:],
                                    cur_psum[0][:],
                                    mybir.ActivationFunctionType.Copy,
                                    scale=dinv_wn_t[:, w : w + 1],
                                )
                                out_cb(w, t1)

                # ---- Stage B: L1 -> h1_dram
                def l1_out(w, t1):
                    t2 = epi.tile([P, HID], dt.float32, tag="t2")
                    nc.vector.tensor_tensor(
                        out=t2[:], in0=t1[:], in1=b1_t[:], op=mybir.AluOpType.add
                    )
                    h1b = epi.tile([P, HID], dt.bfloat16, tag="h1b")
                    nc.scalar.activation(
                        h1b[:], t2[:], mybir.ActivationFunctionType.Relu
                    )
                    nc.sync.dma_start(out=h1_dram[w * P : (w + 1) * P, :], in_=h1b[:])

                if "B" in stages:
                    agg_layer(table1[:, :], T1_ROWS, HID, l1_out)

                if "C" not in stages:
                    nc.gpsimd.dma_start(out=out_ext[:, :], in_=h1_dram[:, :CLS])
                    raise _EndStagesExc()

                # ---- Stage C: table2 shard + AllGather
                h1T = const.tile([HID, SHARD_PAD], dt.bfloat16, tag="h1T")
                nc.sync.dma_start_transpose(h1T[:], h1_dram[:, :])
                for w in range(WINS):
                    ps2 = wpsum.tile([P, CLS], dt.float32, space="PSUM", tag="cps")
                    nc.tensor.matmul(
                        out=ps2[:],
                        lhsT=h1T[:, w * P : (w + 1) * P],
                        rhs=w2_t[:, :],
                        start=True,
                        stop=True,
                    )
                    obc = epi.tile([P, CLS], dt.bfloat16, tag="obc")
                    nc.scalar.activation(
                        obc[:],
                        ps2[:],
                        mybir.ActivationFunctionType.Copy,
                        scale=dinv_wn_t[:, w : w + 1],
                    )
                    rows = min((w + 1) * P, SHARD) - w * P
                    if rows > 0:
                        nc.sync.dma_start(
                            out=ag_in[w * P : w * P + rows, :CLS], in_=obc[:rows, :]
                        )

                nc.gpsimd.collective_compute(
                    "AllGather",
                    mybir.AluOpType.bypass,
                    replica_groups=[list(range(CORES))],
                    ins=[ag_in.ap().opt()],
                    outs=[table2.ap().opt()],
                )

                if "D" not in stages:
                    nc.gpsimd.dma_start(
                        out=out_ext[:, :], in_=table2[:SHARD_PAD, :CLS]
                    )
                    raise _EndStagesExc()

                # ---- Stage D: L2 -> out
                def l2_out(w, t1):
                    t2 = epi.tile([P, CLS], dt.float32, tag="t2o")
                    nc.vector.tensor_tensor(
                        out=t2[:], in0=t1[:], in1=b2_t[:], op=mybir.AluOpType.add
                    )
                    nc.sync.dma_start(out=out_ext[w * P : (w + 1) * P, :], in_=t2[:])

                agg_layer(table2[:, :], N, CLS, l2_out)
          except _EndStagesExc:
            pass

    nc.compile()
    return nc


def _wrap_idx16(idx_flat):
    n = idx_flat.shape[0]
    assert n % 16 == 0
    return idx_flat.reshape(n // 16, 16).T.astype(np.int16).copy()  # [16, n//16]


_CACHE = {}


def _prepare(x, edge_index, W1, b1, W2, b2, repeat=1):
    x = np.asarray(x)
    edge_index = np.asarray(edge_index)
    W1 = np.asarray(W1, dtype=np.float32)
    b1 = np.asarray(b1, dtype=np.float32)
    W2 = np.asarray(W2, dtype=np.float32)
    b2 = np.asarray(b2, dtype=np.float32)

    ekey = hash(edge_index.tobytes())
    if ("pre", ekey) not in _CACHE:
        _CACHE[("pre", ekey)] = _preprocess(edge_index)
    sched, data, dinv = _CACHE[("pre", ekey)]
    key = ("prog", sched["n_idx"], sched["n_tiles"], repeat)
    if key not in _CACHE:
        _CACHE[key] = _build_program(sched, repeat=repeat)
    nc = _CACHE[key]

    xT_full = np.zeros((F, T1_ROWS), dtype=np.float32)
    xT_full[:, :N] = x.T
    xT_bf = _bf16(xT_full)
    dinv_pad = np.zeros(T1_ROWS, dtype=np.float32)
    dinv_pad[:N] = dinv
    iota = np.tile(np.arange(P, dtype=np.float32), (P, 1))
    b1r = np.tile(b1[None, :], (P, 1)).astype(np.float32)
    b2r = np.tile(b2[None, :], (P, 1)).astype(np.float32)

    n_tiles = sched["n_tiles"]
    in_maps = []
    for k in range(CORES):
        idx16, dstoff = data[k]
        dv_loc = np.zeros(SHARD_PAD, dtype=np.float32)
        dv_loc[:SHARD] = dinv[SHARD * k : SHARD * (k + 1)]
        dinv_wn = dv_loc.reshape(WINS, P).T.copy()
        dinv_ck = (
            dinv_pad[SHARD_PAD * k : SHARD_PAD * (k + 1)].reshape(WINS, P).T.copy()
        )
        in_maps.append(
            {
                "xT": np.asarray(xT_bf[:, SHARD_PAD * k : SHARD_PAD * (k + 1)]).copy(),
                "w1": np.asarray(_bf16(W1)),
                "w2": np.asarray(_bf16(W2)),
                "b1r": b1r,
                "b2r": b2r,
                "dinv_ch": dinv_ck,
                "dinv_wn": dinv_wn,
                "iota": iota,
                "idx1": _wrap_idx16(idx16),
                "dstoff": dstoff.reshape(n_tiles, P).T.copy(),
            }
        )
    return nc, in_maps


def kernel(x, edge_index, W1, b1, W2, b2):
    from concourse.bass_utils import run_bass_kernel_spmd

    nc, in_maps = _prepare(x, edge_index, W1, b1, W2, b2)
    res = run_bass_kernel_spmd(nc, in_maps, core_ids=list(range(CORES)), trace=False)
    out = np.empty((N, CLS), dtype=np.float32)
    for k in range(CORES):
        out[SHARD * k : SHARD * (k + 1)] = res.results[k]["out"][:SHARD]
    return out


def make_runner(nc, in_maps):
    """Jit once, stage inputs once; returns fn() -> outputs (blocked)."""
    import jax
    import concourse.mybir as mybir
    from jax.sharding import Mesh, PartitionSpec
    from jax.experimental.shard_map import shard_map
    from concourse.bass2jax import (
        _bass_exec_p,
        install_neuronx_cc_hook,
        partition_id_tensor,
    )

    install_neuronx_cc_hook()
    in_names, out_names, out_avals, zero_outs = [], [], [], []
    for alloc in nc.m.functions[0].allocations:
        if not isinstance(alloc, mybir.MemoryLocationSet):
            continue
        name = alloc.memorylocations[0].name
        if alloc.kind == "ExternalInput":
            if nc.partition_id_tensor is None or name != nc.partition_id_tensor.name:
                in_names.append(name)
        elif alloc.kind == "ExternalOutput":
            out_names.append(name)
            shape = tuple(alloc.tensor_shape)
            dtype = mybir.dt.np(alloc.dtype)
            out_avals.append(jax.core.ShapedArray(shape, dtype))
            zero_outs.append(np.zeros(shape, dtype))
    n_params = len(in_names)
    all_in_names = list(in_names) + out_names
    if nc.partition_id_tensor is not None:
        all_in_names.append(nc.partition_id_tensor.name)

    def _body(*args):
        operands = list(args)
        if nc.partition_id_tensor is not None:
            operands.append(partition_id_tensor())
        return tuple(
            _bass_exec_p.bind(
                *operands,
                out_avals=tuple(out_avals),
                in_names=tuple(all_in_names),
                out_names=tuple(out_names),
                lowering_input_output_aliases=(),
                sim_require_finite=True,
                sim_require_nnan=True,
                nc=nc,
            )
        )

    devices = jax.devices()[: len(in_maps)]
    mesh = Mesh(np.asarray(devices), ("core",))
    specs = (PartitionSpec("core"),) * (n_params + len(out_names))
    fn = jax.jit(
        shard_map(
            _body,
            mesh=mesh,
            in_specs=specs,
            out_specs=(PartitionSpec("core"),) * len(out_names),
            check_rep=False,
        ),
        keep_unused=True,
    )
    args = [
        jax.device_put(np.concatenate([np.asarray(m[n]) for m in in_maps], axis=0))
        for n in in_names
    ]
    args += [
        jax.device_put(np.zeros((len(in_maps) * z.shape[0], *z.shape[1:]), z.dtype))
        for z in zero_outs
    ]

    def run():
        out = fn(*args)
        jax.block_until_ready(out)
        return out

    return run


# revision 13
# speedup vs baseline: 1.3682x; 1.3682x over previous
"""2-layer GCN on 8 Trainium2 NeuronCores (Bass/Tile, SPMD).

Per core (core k owns dst nodes [12500k, 12500(k+1))):
  A) table1 rows [12544k, 12544(k+1)) = dinv * (x @ W1) in bf16 (sharded),
     then AllGather -> full table1 (row == node id within shard blocks,
     padded to 100352 rows).
  B) L1 aggregation: per dst-window (128 nodes) and src-segment (32768 rows,
     int16 dma_gather limit), bulk-gather table1[src] then scatter-add via
     selection matmuls (Sel[e,d] = (dstoff[e]==d)) accumulating in PSUM.
     h1 = relu(dinv * sum + b1) -> DRAM.
  C) table2 shard = dinv * (h1 @ W2) (via dma-transposed h1), AllGather ->
     full table2 with row == node id.
  D) L2 aggregation (same schedule/indices as B): out = dinv * sum + b2.
Host: concat shards.

Note: table1 after stage A has node n at row pad_row(n) = 12544*(n//12544)
+ ... -- rows are just n (cores compute contiguous 12544-row blocks), so
row == node id; rows >= 100000 are junk and never gathered.

Numerics: tables bf16, PSUM accumulate fp32, epilogues fp32.
"""

import numpy as np

N = 100000
F = 128
HID = 64
CLS = 40
CORES = 8
SHARD = 12500
P = 128
WINS = 98  # ceil(12544/128)
SHARD_PAD = WINS * P  # 12544
SEG = 32768
NSEG = 4
ELEM1 = 128  # bf16 elems per table row = 256B (real payload in first cols)
T1_ROWS = SHARD_PAD * CORES  # 100352


def _bf16(a):
    import ml_dtypes

    return np.asarray(a, dtype=ml_dtypes.bfloat16)


def _preprocess(edge_index):
    src = np.concatenate([edge_index[0].astype(np.int64), np.arange(N, dtype=np.int64)])
    dst = np.concatenate([edge_index[1].astype(np.int64), np.arange(N, dtype=np.int64)])
    deg = np.bincount(dst, minlength=N).astype(np.float64)
    dinv = np.where(deg > 0, 1.0 / np.sqrt(deg), 0.0).astype(np.float32)

    per_core = []
    counts = np.zeros((CORES, WINS, NSEG), dtype=np.int64)
    for k in range(CORES):
        m = (dst >= SHARD * k) & (dst < SHARD * (k + 1))
        s_k = src[m]
        local = dst[m] - SHARD * k
        win = local // P
        off = local % P
        seg = s_k // SEG
        order = np.lexsort((seg, win))
        s_k, win, off, seg = s_k[order], win[order], off[order], seg[order]
        np.add.at(counts[k], (win, seg), 1)
        per_core.append((s_k, win, off, seg))

    gmax = counts.max(axis=0)  # [WINS, NSEG]
    gm16 = ((gmax + 15) // 16) * 16  # idx counts per (w,s) call, common

    calls = []  # (w, s, idx_off16, num_idxs, n_groups, tile_base)
    idx_off = 0
    n_tiles = 0
    for w in range(WINS):
        for s in range(NSEG):
            ni = int(gm16[w, s])
            if ni == 0:
                continue
            ng = (ni + P - 1) // P
            assert ni <= 1024, f"gather call too large: {ni}"
            calls.append((w, s, idx_off // 16, ni, ng, n_tiles))
            idx_off += ni
            n_tiles += ng
    n_idx = idx_off
    assert n_idx % 16 == 0

    first_tile = {}
    last_tile = {}
    t = 0
    for w, s, _, _, ng, tb in calls:
        for g in range(ng):
            if w not in first_tile:
                first_tile[w] = t
            last_tile[w] = t
            t += 1

    data = []
    for k in range(CORES):
        s_k, win, off, seg = per_core[k]
        idx16 = np.zeros(n_idx, dtype=np.int16)
        dstoff = np.full(n_tiles * P, -1.0, dtype=np.float32)
        key = win * NSEG + seg
        starts = np.concatenate(([0], np.flatnonzero(np.diff(key)) + 1))
        ends = np.concatenate((starts[1:], [len(key)]))
        bounds = {int(key[st]): (int(st), int(en)) for st, en in zip(starts, ends)}
        for w, s, off16, ni, ng, tb in calls:
            i, j = bounds.get(w * NSEG + s, (0, 0))
            n_e = j - i
            st = off16 * 16
            if n_e > 0:
                idx16[st : st + n_e] = (s_k[i:j] - SEG * s).astype(np.int16)
                dstoff[tb * P : tb * P + n_e] = off[i:j].astype(np.float32)
        data.append((idx16, dstoff))

    sched = {
        "calls": calls,
        "n_idx": n_idx,
        "n_tiles": n_tiles,
        "first_tile": first_tile,
        "last_tile": last_tile,
    }
    return sched, data, dinv


class _EndStagesExc(Exception):
    pass


def _build_program(sched, stages="ABCD", repeat=1):
    import concourse.mybir as mybir
    import concourse.tile as tile
    from concourse import bacc
    from concourse import library_config

    calls = sched["calls"]
    n_idx = sched["n_idx"]
    n_tiles = sched["n_tiles"]
    first_tile = sched["first_tile"]
    last_tile = sched["last_tile"]
    max_ng = max(c[4] for c in calls)

    nc = bacc.Bacc("TRN2", target_bir_lowering=False, debug=False, num_devices=CORES, num_swdge_queues=4)
    dt = mybir.dt

    xT = nc.dram_tensor("xT", [P, SHARD_PAD], dt.bfloat16, kind="ExternalInput")
    w1 = nc.dram_tensor("w1", [F, HID], dt.bfloat16, kind="ExternalInput")
    w2 = nc.dram_tensor("w2", [HID, CLS], dt.bfloat16, kind="ExternalInput")
    b1r = nc.dram_tensor("b1r", [P, HID], dt.float32, kind="ExternalInput")
    b2r = nc.dram_tensor("b2r", [P, CLS], dt.float32, kind="ExternalInput")
    dinv_ch = nc.dram_tensor("dinv_ch", [P, WINS], dt.float32, kind="ExternalInput")
    dinv_wn = nc.dram_tensor("dinv_wn", [P, WINS], dt.float32, kind="ExternalInput")
    iota_in = nc.dram_tensor("iota", [P, P], dt.float32, kind="ExternalInput")
    idx1_in = nc.dram_tensor("idx1", [16, n_idx // 16], dt.int16, kind="ExternalInput")
    dstoff_in = nc.dram_tensor("dstoff", [P, n_tiles], dt.float32, kind="ExternalInput")
    out_ext = nc.dram_tensor("out", [SHARD_PAD, CLS], dt.float32, kind="ExternalOutput")

    table1 = nc.dram_tensor(
        "table1", [T1_ROWS, ELEM1], dt.bfloat16, addr_space="Shared"
    )
    ag1_in = nc.dram_tensor("ag1_in", [SHARD_PAD, ELEM1], dt.bfloat16)
    h1_dram = nc.dram_tensor("h1_dram", [SHARD_PAD, HID], dt.bfloat16)
    ag_in = nc.dram_tensor("ag_in", [SHARD, ELEM1], dt.bfloat16)
    table2 = nc.dram_tensor("table2", [N, ELEM1], dt.bfloat16, addr_space="Shared")

    with tile.TileContext(nc) as tc:
        with (
            tc.tile_pool(name="const", bufs=1) as const,
            tc.tile_pool(name="xs", bufs=3) as xs,
            tc.tile_pool(name="apsum", bufs=2, space="PSUM") as apsum,
            tc.tile_pool(name="aout", bufs=3) as aout,
            tc.tile_pool(name="msg", bufs=8) as msgp,
            tc.tile_pool(name="sel", bufs=6) as selp,
            tc.tile_pool(name="wpsum", bufs=2, space="PSUM") as wpsum,
            tc.tile_pool(name="epi", bufs=6) as epi,
        ):
          try:
            nc.gpsimd.load_library(library_config.mlp)

            w1_t = const.tile([F, HID], dt.bfloat16)
            nc.sync.dma_start(out=w1_t[:], in_=w1[:, :])
            w2_t = const.tile([HID, CLS], dt.bfloat16)
            nc.sync.dma_start(out=w2_t[:], in_=w2[:, :])
            b1_t = const.tile([P, HID], dt.float32)
            nc.sync.dma_start(out=b1_t[:], in_=b1r[:, :])
            b2_t = const.tile([P, CLS], dt.float32)
            nc.sync.dma_start(out=b2_t[:], in_=b2r[:, :])
            dinv_ch_t = const.tile([P, WINS], dt.float32)
            nc.sync.dma_start(out=dinv_ch_t[:], in_=dinv_ch[:, :])
            dinv_wn_t = const.tile([P, WINS], dt.float32)
            nc.sync.dma_start(out=dinv_wn_t[:], in_=dinv_wn[:, :])
            iota_t = const.tile([P, P], dt.float32)
            nc.sync.dma_start(out=iota_t[:], in_=iota_in[:, :])
            idx1_t = const.tile([P, n_idx // 16], dt.int16)
            for r in range(8):
                nc.sync.dma_start(
                    out=idx1_t[16 * r : 16 * (r + 1), :], in_=idx1_in[:, :]
                )
            dstoff_t = const.tile([P, n_tiles], dt.float32)
            nc.sync.dma_start(out=dstoff_t[:], in_=dstoff_in[:, :])

            for _rep in range(repeat):
                # ---- Stage A: own table1 rows = dinv * (x @ W1); AllGather
                GB = 4
                for cb in range(0, WINS, GB):
                    nb = min(GB, WINS - cb)
                    xt = xs.tile([P, GB * P], dt.bfloat16, tag="xt")
                    nc.sync.dma_start(
                        out=xt[:, : nb * P], in_=xT[:, cb * P : (cb + nb) * P]
                    )
                    ps = apsum.tile([P, GB * HID], dt.float32, space="PSUM", tag="aps")
                    ob = aout.tile([P, GB * ELEM1], dt.bfloat16, tag="ob")
                    for j in range(nb):
                        nc.tensor.matmul(
                            out=ps[:, j * HID : (j + 1) * HID],
                            lhsT=xt[:, j * P : (j + 1) * P],
                            rhs=w1_t[:, :],
                            start=True,
                            stop=True,
                        )
                        if ((cb // GB) + j) % 2 == 0:
                            nc.scalar.activation(
                                ob[:, j * ELEM1 : j * ELEM1 + HID],
                                ps[:, j * HID : (j + 1) * HID],
                                mybir.ActivationFunctionType.Copy,
                                scale=dinv_ch_t[:, cb + j : cb + j + 1],
                            )
                        else:
                            nc.vector.tensor_scalar_mul(
                                ob[:, j * ELEM1 : j * ELEM1 + HID],
                                ps[:, j * HID : (j + 1) * HID],
                                dinv_ch_t[:, cb + j : cb + j + 1],
                            )
                    nc.sync.dma_start(
                        out=ag1_in[:, :].rearrange("(c p) e -> p c e", p=P)[
                            :, cb : cb + nb, :
                        ],
                        in_=ob[:, : nb * ELEM1].rearrange("p (c e) -> p c e", e=ELEM1),
                    )

                if "A2" not in stages:
                    nc.gpsimd.collective_compute(
                        "AllGather",
                        mybir.AluOpType.bypass,
                        replica_groups=[list(range(CORES))],
                        ins=[ag1_in.ap().opt()],
                        outs=[table1.ap().opt()],
                    )

                # ---- shared aggregation layer
                def agg_layer(table_ap, table_rows, d_out, out_cb):
                    cur_psum = [None]
                    for ci, (w, s, off16, ni, ng, tb) in enumerate(calls):
                        mt = msgp.tile([P, max_ng * ELEM1], dt.bfloat16, tag="mt")
                        if ci < 8 and _rep == 0:
                            nc.vector.memset(mt[:], 0.0)
                        nc.gpsimd.dma_gather(
                            out_ap=mt[:, : ng * ELEM1].rearrange(
                                "p (g e) -> p g e", e=ELEM1
                            ),
                            in_ap=table_ap[SEG * s : min(SEG * (s + 1), table_rows), :],
                            idxs_ap=idx1_t[:, off16 : off16 + ni // 16],
                            num_idxs=ni,
                            num_idxs_reg=ni,
                            elem_size=ELEM1,
                            queue_num=0,
                        )
                        for g in range(ng):
                            t = tb + g
                            if t == first_tile[w]:
                                wps = wpsum.tile(
                                    [P, d_out], dt.float32, space="PSUM", tag="wps"
                                )
                                cur_psum[0] = wps
                            sel = selp.tile([P, P], dt.bfloat16, tag="sel")
                            nc.vector.tensor_scalar(
                                sel[:],
                                iota_t[:],
                                dstoff_t[:, t : t + 1],
                                None,
                                mybir.AluOpType.is_equal,
                            )
                            nc.tensor.matmul(
                                out=cur_psum[0][:],
                                lhsT=sel[:],
                                rhs=mt[:, g * ELEM1 : g * ELEM1 + d_out],
                                start=(t == first_tile[w]),
                                stop=(t == last_tile[w]),
                            )
                            if t == last_tile[w]:
                                t1 = epi.tile([P, d_out], dt.float32, tag="t1")
                                nc.scalar.activation(
                                    t1[:],
                                    cur_psum[0][:],
                                    mybir.ActivationFunctionType.Copy,
                                    scale=dinv_wn_t[:, w : w + 1],
                                )
                                out_cb(w, t1)

                # ---- Stage B: L1 -> h1_dram
                def l1_out(w, t1):
                    t2 = epi.tile([P, HID], dt.float32, tag="t2")
                    nc.vector.tensor_tensor(
                        out=t2[:], in0=t1[:], in1=b1_t[:], op=mybir.AluOpType.add
                    )
                    h1b = epi.tile([P, HID], dt.bfloat16, tag="h1b")
                    nc.scalar.activation(
                        h1b[:], t2[:], mybir.ActivationFunctionType.Relu
                    )
                    nc.sync.dma_start(out=h1_dram[w * P : (w + 1) * P, :], in_=h1b[:])

                if "B" in stages:
                    agg_layer(table1[:, :], T1_ROWS, HID, l1_out)

                if "C" not in stages:
                    nc.gpsimd.dma_start(out=out_ext[:, :], in_=h1_dram[:, :CLS])
                    raise _EndStagesExc()

                # ---- Stage C: table2 shard + AllGather
                h1T = const.tile([HID, SHARD_PAD], dt.bfloat16, tag="h1T")
                nc.sync.dma_start_transpose(h1T[:], h1_dram[:, :])
                for w in range(WINS):
                    ps2 = wpsum.tile([P, CLS], dt.float32, space="PSUM", tag="cps")
                    nc.tensor.matmul(
                        out=ps2[:],
                        lhsT=h1T[:, w * P : (w + 1) * P],
                        rhs=w2_t[:, :],
                        start=True,
                        stop=True,
                    )
                    obc = epi.tile([P, CLS], dt.bfloat16, tag="obc")
                    nc.scalar.activation(
                        obc[:],
                        ps2[:],
                        mybir.ActivationFunctionType.Copy,
                        scale=dinv_wn_t[:, w : w + 1],
                    )
                    rows = min((w + 1) * P, SHARD) - w * P
                    if rows > 0:
                        nc.sync.dma_start(
                            out=ag_in[w * P : w * P + rows, :CLS], in_=obc[:rows, :]
                        )

                nc.gpsimd.collective_compute(
                    "AllGather",
                    mybir.AluOpType.bypass,
                    replica_groups=[list(range(CORES))],
                    ins=[ag_in.ap().opt()],
                    outs=[table2.ap().opt()],
                )

                if "D" not in stages:
                    nc.gpsimd.dma_start(
                        out=out_ext[:, :], in_=table2[:SHARD_PAD, :CLS]
                    )
                    raise _EndStagesExc()

                # ---- Stage D: L2 -> out
                def l2_out(w, t1):
                    t2 = epi.tile([P, CLS], dt.float32, tag="t2o")
                    nc.vector.tensor_tensor(
                        out=t2[:], in0=t1[:], in1=b2_t[:], op=mybir.AluOpType.add
                    )
                    nc.sync.dma_start(out=out_ext[w * P : (w + 1) * P, :], in_=t2[:])

                agg_layer(table2[:, :], N, CLS, l2_out)
          except _EndStagesExc:
            pass

    nc.compile()
    return nc


def _wrap_idx16(idx_flat):
    n = idx_flat.shape[0]
    assert n % 16 == 0
    return idx_flat.reshape(n // 16, 16).T.astype(np.int16).copy()  # [16, n//16]


_CACHE = {}


def _prepare(x, edge_index, W1, b1, W2, b2, repeat=1):
    x = np.asarray(x)
    edge_index = np.asarray(edge_index)
    W1 = np.asarray(W1, dtype=np.float32)
    b1 = np.asarray(b1, dtype=np.float32)
    W2 = np.asarray(W2, dtype=np.float32)
    b2 = np.asarray(b2, dtype=np.float32)

    ekey = hash(edge_index.tobytes())
    if ("pre", ekey) not in _CACHE:
        _CACHE[("pre", ekey)] = _preprocess(edge_index)
    sched, data, dinv = _CACHE[("pre", ekey)]
    key = ("prog", sched["n_idx"], sched["n_tiles"], repeat)
    if key not in _CACHE:
        _CACHE[key] = _build_program(sched, repeat=repeat)
    nc = _CACHE[key]

    xT_full = np.zeros((F, T1_ROWS), dtype=np.float32)
    xT_full[:, :N] = x.T
    xT_bf = _bf16(xT_full)
    dinv_pad = np.zeros(T1_ROWS, dtype=np.float32)
    dinv_pad[:N] = dinv
    iota = np.tile(np.arange(P, dtype=np.float32), (P, 1))
    b1r = np.tile(b1[None, :], (P, 1)).astype(np.float32)
    b2r = np.tile(b2[None, :], (P, 1)).astype(np.float32)

    n_tiles = sched["n_tiles"]
    in_maps = []
    for k in range(CORES):
        idx16, dstoff = data[k]
        dv_loc = np.zeros(SHARD_PAD, dtype=np.float32)
        dv_loc[:SHARD] = dinv[SHARD * k : SHARD * (k + 1)]
        dinv_wn = dv_loc.reshape(WINS, P).T.copy()
        dinv_ck = (
            dinv_pad[SHARD_PAD * k : SHARD_PAD * (k + 1)].reshape(WINS, P).T.copy()
        )
        in_maps.append(
            {
                "xT": np.asarray(xT_bf[:, SHARD_PAD * k : SHARD_PAD * (k + 1)]).copy(),
                "w1": np.asarray(_bf16(W1)),
                "w2": np.asarray(_bf16(W2)),
                "b1r": b1r,
                "b2r": b2r,
                "dinv_ch": dinv_ck,
                "dinv_wn": dinv_wn,
                "iota": iota,
                "idx1": _wrap_idx16(idx16),
                "dstoff": dstoff.reshape(n_tiles, P).T.copy(),
            }
        )
    return nc, in_maps


def kernel(x, edge_index, W1, b1, W2, b2):
    from concourse.bass_utils import run_bass_kernel_spmd

    nc, in_maps = _prepare(x, edge_index, W1, b1, W2, b2)
    res = run_bass_kernel_spmd(nc, in_maps, core_ids=list(range(CORES)), trace=False)
    out = np.empty((N, CLS), dtype=np.float32)
    for k in range(CORES):
        out[SHARD * k : SHARD * (k + 1)] = res.results[k]["out"][:SHARD]
    return out


def make_runner(nc, in_maps):
    """Jit once, stage inputs once; returns fn() -> outputs (blocked)."""
    import jax
    import concourse.mybir as mybir
    from jax.sharding import Mesh, PartitionSpec
    from jax.experimental.shard_map import shard_map
    from concourse.bass2jax import (
        _bass_exec_p,
        install_neuronx_cc_hook,
        partition_id_tensor,
    )

    install_neuronx_cc_hook()
    in_names, out_names, out_avals, zero_outs = [], [], [], []
    for alloc in nc.m.functions[0].allocations:
        if not isinstance(alloc, mybir.MemoryLocationSet):
            continue
        name = alloc.memorylocations[0].name
        if alloc.kind == "ExternalInput":
            if nc.partition_id_tensor is None or name != nc.partition_id_tensor.name:
                in_names.append(name)
        elif alloc.kind == "ExternalOutput":
            out_names.append(name)
            shape = tuple(alloc.tensor_shape)
            dtype = mybir.dt.np(alloc.dtype)
            out_avals.append(jax.core.ShapedArray(shape, dtype))
            zero_outs.append(np.zeros(shape, dtype))
    n_params = len(in_names)
    all_in_names = list(in_names) + out_names
    if nc.partition_id_tensor is not None:
        all_in_names.append(nc.partition_id_tensor.name)

    def _body(*args):
        operands = list(args)
        if nc.partition_id_tensor is not None:
            operands.append(partition_id_tensor())
        return tuple(
            _bass_exec_p.bind(
                *operands,
                out_avals=tuple(out_avals),
                in_names=tuple(all_in_names),
                out_names=tuple(out_names),
                lowering_input_output_aliases=(),
                sim_require_finite=True,
                sim_require_nnan=True,
                nc=nc,
            )
        )

    devices = jax.devices()[: len(in_maps)]
    mesh = Mesh(np.asarray(devices), ("core",))
    specs = (PartitionSpec("core"),) * (n_params + len(out_names))
    fn = jax.jit(
        shard_map(
            _body,
            mesh=mesh,
            in_specs=specs,
            out_specs=(PartitionSpec("core"),) * len(out_names),
            check_rep=False,
        ),
        keep_unused=True,
    )
    args = [
        jax.device_put(np.concatenate([np.asarray(m[n]) for m in in_maps], axis=0))
        for n in in_names
    ]
    args += [
        jax.device_put(np.zeros((len(in_maps) * z.shape[0], *z.shape[1:]), z.dtype))
        for z in zero_outs
    ]

    def run():
        out = fn(*args)
        jax.block_until_ready(out)
        return out

    return run


# revision 14
# speedup vs baseline: 2.9463x; 2.1534x over previous
"""2-layer GCN on 8 Trainium2 NeuronCores (Bass/Tile, SPMD).

Per core (core k owns dst nodes [12500k, 12500(k+1))):
  A) table1 rows [12544k, 12544(k+1)) = dinv * (x @ W1) in bf16 (sharded),
     then AllGather -> full table1 (row == node id within shard blocks,
     padded to 100352 rows).
  B) L1 aggregation: per dst-window (128 nodes) and src-segment (32768 rows,
     int16 dma_gather limit), bulk-gather table1[src] then scatter-add via
     selection matmuls (Sel[e,d] = (dstoff[e]==d)) accumulating in PSUM.
     h1 = relu(dinv * sum + b1) -> DRAM.
  C) table2 shard = dinv * (h1 @ W2) (via dma-transposed h1), AllGather ->
     full table2 with row == node id.
  D) L2 aggregation (same schedule/indices as B): out = dinv * sum + b2.
Host: concat shards.

Note: table1 after stage A has node n at row pad_row(n) = 12544*(n//12544)
+ ... -- rows are just n (cores compute contiguous 12544-row blocks), so
row == node id; rows >= 100000 are junk and never gathered.

Numerics: tables bf16, PSUM accumulate fp32, epilogues fp32.
"""

import numpy as np

N = 100000
F = 128
HID = 64
CLS = 40
CORES = 8
SHARD = 12500
P = 128
WINS = 98  # ceil(12544/128)
SHARD_PAD = WINS * P  # 12544
SEG = 32768
NSEG = 4
ELEM1 = 128  # bf16 elems per table row = 256B (real payload in first cols)
T1_ROWS = SHARD_PAD * CORES  # 100352


def _bf16(a):
    import ml_dtypes

    return np.asarray(a, dtype=ml_dtypes.bfloat16)


def _preprocess(edge_index):
    src = np.concatenate([edge_index[0].astype(np.int64), np.arange(N, dtype=np.int64)])
    dst = np.concatenate([edge_index[1].astype(np.int64), np.arange(N, dtype=np.int64)])
    deg = np.bincount(dst, minlength=N).astype(np.float64)
    dinv = np.where(deg > 0, 1.0 / np.sqrt(deg), 0.0).astype(np.float32)

    per_core = []
    counts = np.zeros((CORES, WINS, NSEG), dtype=np.int64)
    for k in range(CORES):
        m = (dst >= SHARD * k) & (dst < SHARD * (k + 1))
        s_k = src[m]
        local = dst[m] - SHARD * k
        win = local // P
        off = local % P
        seg = s_k // SEG
        order = np.lexsort((seg, win))
        s_k, win, off, seg = s_k[order], win[order], off[order], seg[order]
        np.add.at(counts[k], (win, seg), 1)
        per_core.append((s_k, win, off, seg))

    gmax = counts.max(axis=0)  # [WINS, NSEG]
    gm16 = ((gmax + 15) // 16) * 16  # idx counts per (w,s) call, common

    calls = []  # (w, s, idx_off16, num_idxs, n_groups, tile_base)
    idx_off = 0
    n_tiles = 0
    for w in range(WINS):
        for s in range(NSEG):
            ni = int(gm16[w, s])
            if ni == 0:
                continue
            ng = (ni + P - 1) // P
            assert ni <= 1024, f"gather call too large: {ni}"
            calls.append((w, s, idx_off // 16, ni, ng, n_tiles))
            idx_off += ni
            n_tiles += ng
    n_idx = idx_off
    assert n_idx % 16 == 0

    first_tile = {}
    last_tile = {}
    t = 0
    for w, s, _, _, ng, tb in calls:
        for g in range(ng):
            if w not in first_tile:
                first_tile[w] = t
            last_tile[w] = t
            t += 1

    data = []
    for k in range(CORES):
        s_k, win, off, seg = per_core[k]
        idx16 = np.zeros(n_idx, dtype=np.int16)
        dstoff = np.full(n_tiles * P, -1.0, dtype=np.float32)
        key = win * NSEG + seg
        starts = np.concatenate(([0], np.flatnonzero(np.diff(key)) + 1))
        ends = np.concatenate((starts[1:], [len(key)]))
        bounds = {int(key[st]): (int(st), int(en)) for st, en in zip(starts, ends)}
        for w, s, off16, ni, ng, tb in calls:
            i, j = bounds.get(w * NSEG + s, (0, 0))
            n_e = j - i
            st = off16 * 16
            if n_e > 0:
                idx16[st : st + n_e] = (s_k[i:j] - SEG * s).astype(np.int16)
                dstoff[tb * P : tb * P + n_e] = off[i:j].astype(np.float32)
        data.append((idx16, dstoff))

    sched = {
        "calls": calls,
        "n_idx": n_idx,
        "n_tiles": n_tiles,
        "first_tile": first_tile,
        "last_tile": last_tile,
    }
    return sched, data, dinv


class _EndStagesExc(Exception):
    pass


def _build_program(sched, stages="ABCD", repeat=1):
    import concourse.mybir as mybir
    import concourse.tile as tile
    from concourse import bacc
    from concourse import library_config

    calls = sched["calls"]
    n_idx = sched["n_idx"]
    n_tiles = sched["n_tiles"]
    first_tile = sched["first_tile"]
    last_tile = sched["last_tile"]
    max_ng = max(c[4] for c in calls)

    nc = bacc.Bacc("TRN2", target_bir_lowering=False, debug=False, num_devices=CORES, num_swdge_queues=4, dynamic_dma_scratch_size=65536)
    dt = mybir.dt

    xT = nc.dram_tensor("xT", [P, SHARD_PAD], dt.bfloat16, kind="ExternalInput")
    w1 = nc.dram_tensor("w1", [F, HID], dt.bfloat16, kind="ExternalInput")
    w2 = nc.dram_tensor("w2", [HID, CLS], dt.bfloat16, kind="ExternalInput")
    b1r = nc.dram_tensor("b1r", [P, HID], dt.float32, kind="ExternalInput")
    b2r = nc.dram_tensor("b2r", [P, CLS], dt.float32, kind="ExternalInput")
    dinv_ch = nc.dram_tensor("dinv_ch", [P, WINS], dt.float32, kind="ExternalInput")
    dinv_wn = nc.dram_tensor("dinv_wn", [P, WINS], dt.float32, kind="ExternalInput")
    iota_in = nc.dram_tensor("iota", [P, P], dt.float32, kind="ExternalInput")
    idx1_in = nc.dram_tensor("idx1", [16, n_idx // 16], dt.int16, kind="ExternalInput")
    dstoff_in = nc.dram_tensor("dstoff", [P, n_tiles], dt.float32, kind="ExternalInput")
    out_ext = nc.dram_tensor("out", [SHARD_PAD, CLS], dt.float32, kind="ExternalOutput")

    table1 = nc.dram_tensor(
        "table1", [T1_ROWS, ELEM1], dt.bfloat16, addr_space="Shared"
    )
    ag1_in = nc.dram_tensor("ag1_in", [SHARD_PAD, ELEM1], dt.bfloat16)
    h1_dram = nc.dram_tensor("h1_dram", [SHARD_PAD, HID], dt.bfloat16)
    ag_in = nc.dram_tensor("ag_in", [SHARD, ELEM1], dt.bfloat16)
    table2 = nc.dram_tensor("table2", [N, ELEM1], dt.bfloat16, addr_space="Shared")

    with tile.TileContext(nc) as tc:
        with (
            tc.tile_pool(name="const", bufs=1) as const,
            tc.tile_pool(name="xs", bufs=3) as xs,
            tc.tile_pool(name="apsum", bufs=2, space="PSUM") as apsum,
            tc.tile_pool(name="aout", bufs=3) as aout,
            tc.tile_pool(name="msg", bufs=8) as msgp,
            tc.tile_pool(name="sel", bufs=6) as selp,
            tc.tile_pool(name="wpsum", bufs=2, space="PSUM") as wpsum,
            tc.tile_pool(name="epi", bufs=6) as epi,
        ):
          try:
            nc.gpsimd.load_library(library_config.mlp)

            w1_t = const.tile([F, HID], dt.bfloat16)
            nc.sync.dma_start(out=w1_t[:], in_=w1[:, :])
            w2_t = const.tile([HID, CLS], dt.bfloat16)
            nc.sync.dma_start(out=w2_t[:], in_=w2[:, :])
            b1_t = const.tile([P, HID], dt.float32)
            nc.sync.dma_start(out=b1_t[:], in_=b1r[:, :])
            b2_t = const.tile([P, CLS], dt.float32)
            nc.sync.dma_start(out=b2_t[:], in_=b2r[:, :])
            dinv_ch_t = const.tile([P, WINS], dt.float32)
            nc.sync.dma_start(out=dinv_ch_t[:], in_=dinv_ch[:, :])
            dinv_wn_t = const.tile([P, WINS], dt.float32)
            nc.sync.dma_start(out=dinv_wn_t[:], in_=dinv_wn[:, :])
            iota_t = const.tile([P, P], dt.float32)
            nc.sync.dma_start(out=iota_t[:], in_=iota_in[:, :])
            idx1_t = const.tile([P, n_idx // 16], dt.int16)
            for r in range(8):
                nc.sync.dma_start(
                    out=idx1_t[16 * r : 16 * (r + 1), :], in_=idx1_in[:, :]
                )
            dstoff_t = const.tile([P, n_tiles], dt.float32)
            nc.sync.dma_start(out=dstoff_t[:], in_=dstoff_in[:, :])

            for _rep in range(repeat):
                # ---- Stage A: own table1 rows = dinv * (x @ W1); AllGather
                GB = 4
                for cb in range(0, WINS, GB):
                    nb = min(GB, WINS - cb)
                    xt = xs.tile([P, GB * P], dt.bfloat16, tag="xt")
                    nc.sync.dma_start(
                        out=xt[:, : nb * P], in_=xT[:, cb * P : (cb + nb) * P]
                    )
                    ps = apsum.tile([P, GB * HID], dt.float32, space="PSUM", tag="aps")
                    ob = aout.tile([P, GB * ELEM1], dt.bfloat16, tag="ob")
                    for j in range(nb):
                        nc.tensor.matmul(
                            out=ps[:, j * HID : (j + 1) * HID],
                            lhsT=xt[:, j * P : (j + 1) * P],
                            rhs=w1_t[:, :],
                            start=True,
                            stop=True,
                        )
                        if ((cb // GB) + j) % 2 == 0:
                            nc.scalar.activation(
                                ob[:, j * ELEM1 : j * ELEM1 + HID],
                                ps[:, j * HID : (j + 1) * HID],
                                mybir.ActivationFunctionType.Copy,
                                scale=dinv_ch_t[:, cb + j : cb + j + 1],
                            )
                        else:
                            nc.vector.tensor_scalar_mul(
                                ob[:, j * ELEM1 : j * ELEM1 + HID],
                                ps[:, j * HID : (j + 1) * HID],
                                dinv_ch_t[:, cb + j : cb + j + 1],
                            )
                    nc.sync.dma_start(
                        out=ag1_in[:, :].rearrange("(c p) e -> p c e", p=P)[
                            :, cb : cb + nb, :
                        ],
                        in_=ob[:, : nb * ELEM1].rearrange("p (c e) -> p c e", e=ELEM1),
                    )

                if "A2" not in stages:
                    nc.gpsimd.collective_compute(
                        "AllGather",
                        mybir.AluOpType.bypass,
                        replica_groups=[list(range(CORES))],
                        ins=[ag1_in.ap().opt()],
                        outs=[table1.ap().opt()],
                    )

                # ---- shared aggregation layer
                def agg_layer(table_ap, table_rows, d_out, out_cb):
                    cur_psum = [None]
                    for ci, (w, s, off16, ni, ng, tb) in enumerate(calls):
                        mt = msgp.tile([P, max_ng * ELEM1], dt.bfloat16, tag="mt")
                        if ci < 8 and _rep == 0:
                            nc.vector.memset(mt[:], 0.0)
                        nc.gpsimd.dma_gather(
                            out_ap=mt[:, : ng * ELEM1].rearrange(
                                "p (g e) -> p g e", e=ELEM1
                            ),
                            in_ap=table_ap[SEG * s : min(SEG * (s + 1), table_rows), :],
                            idxs_ap=idx1_t[:, off16 : off16 + ni // 16],
                            num_idxs=ni,
                            num_idxs_reg=ni,
                            elem_size=ELEM1,
                            queue_num=ci % 4,
                        )
                        for g in range(ng):
                            t = tb + g
                            if t == first_tile[w]:
                                wps = wpsum.tile(
                                    [P, d_out], dt.float32, space="PSUM", tag="wps"
                                )
                                cur_psum[0] = wps
                            sel = selp.tile([P, P], dt.bfloat16, tag="sel")
                            nc.vector.tensor_scalar(
                                sel[:],
                                iota_t[:],
                                dstoff_t[:, t : t + 1],
                                None,
                                mybir.AluOpType.is_equal,
                            )
                            nc.tensor.matmul(
                                out=cur_psum[0][:],
                                lhsT=sel[:],
                                rhs=mt[:, g * ELEM1 : g * ELEM1 + d_out],
                                start=(t == first_tile[w]),
                                stop=(t == last_tile[w]),
                            )
                            if t == last_tile[w]:
                                t1 = epi.tile([P, d_out], dt.float32, tag="t1")
                                nc.scalar.activation(
                                    t1[:],
                                    cur_psum[0][:],
                                    mybir.ActivationFunctionType.Copy,
                                    scale=dinv_wn_t[:, w : w + 1],
                                )
                                out_cb(w, t1)

                # ---- Stage B: L1 -> h1_dram
                def l1_out(w, t1):
                    t2 = epi.tile([P, HID], dt.float32, tag="t2")
                    nc.vector.tensor_tensor(
                        out=t2[:], in0=t1[:], in1=b1_t[:], op=mybir.AluOpType.add
                    )
                    h1b = epi.tile([P, HID], dt.bfloat16, tag="h1b")
                    nc.scalar.activation(
                        h1b[:], t2[:], mybir.ActivationFunctionType.Relu
                    )
                    nc.sync.dma_start(out=h1_dram[w * P : (w + 1) * P, :], in_=h1b[:])

                if "B" in stages:
                    agg_layer(table1[:, :], T1_ROWS, HID, l1_out)

                if "C" not in stages:
                    nc.gpsimd.dma_start(out=out_ext[:, :], in_=h1_dram[:, :CLS])
                    raise _EndStagesExc()

                # ---- Stage C: table2 shard + AllGather
                h1T = const.tile([HID, SHARD_PAD], dt.bfloat16, tag="h1T")
                nc.sync.dma_start_transpose(h1T[:], h1_dram[:, :])
                for w in range(WINS):
                    ps2 = wpsum.tile([P, CLS], dt.float32, space="PSUM", tag="cps")
                    nc.tensor.matmul(
                        out=ps2[:],
                        lhsT=h1T[:, w * P : (w + 1) * P],
                        rhs=w2_t[:, :],
                        start=True,
                        stop=True,
                    )
                    obc = epi.tile([P, CLS], dt.bfloat16, tag="obc")
                    nc.scalar.activation(
                        obc[:],
                        ps2[:],
                        mybir.ActivationFunctionType.Copy,
                        scale=dinv_wn_t[:, w : w + 1],
                    )
                    rows = min((w + 1) * P, SHARD) - w * P
                    if rows > 0:
                        nc.sync.dma_start(
                            out=ag_in[w * P : w * P + rows, :CLS], in_=obc[:rows, :]
                        )

                nc.gpsimd.collective_compute(
                    "AllGather",
                    mybir.AluOpType.bypass,
                    replica_groups=[list(range(CORES))],
                    ins=[ag_in.ap().opt()],
                    outs=[table2.ap().opt()],
                )

                if "D" not in stages:
                    nc.gpsimd.dma_start(
                        out=out_ext[:, :], in_=table2[:SHARD_PAD, :CLS]
                    )
                    raise _EndStagesExc()

                # ---- Stage D: L2 -> out
                def l2_out(w, t1):
                    t2 = epi.tile([P, CLS], dt.float32, tag="t2o")
                    nc.vector.tensor_tensor(
                        out=t2[:], in0=t1[:], in1=b2_t[:], op=mybir.AluOpType.add
                    )
                    nc.sync.dma_start(out=out_ext[w * P : (w + 1) * P, :], in_=t2[:])

                agg_layer(table2[:, :], N, CLS, l2_out)
          except _EndStagesExc:
            pass

    nc.compile()
    return nc


def _wrap_idx16(idx_flat):
    n = idx_flat.shape[0]
    assert n % 16 == 0
    return idx_flat.reshape(n // 16, 16).T.astype(np.int16).copy()  # [16, n//16]


_CACHE = {}


def _prepare(x, edge_index, W1, b1, W2, b2, repeat=1):
    x = np.asarray(x)
    edge_index = np.asarray(edge_index)
    W1 = np.asarray(W1, dtype=np.float32)
    b1 = np.asarray(b1, dtype=np.float32)
    W2 = np.asarray(W2, dtype=np.float32)
    b2 = np.asarray(b2, dtype=np.float32)

    ekey = hash(edge_index.tobytes())
    if ("pre", ekey) not in _CACHE:
        _CACHE[("pre", ekey)] = _preprocess(edge_index)
    sched, data, dinv = _CACHE[("pre", ekey)]
    key = ("prog", sched["n_idx"], sched["n_tiles"], repeat)
    if key not in _CACHE:
        _CACHE[key] = _build_program(sched, repeat=repeat)
    nc = _CACHE[key]

    xT_full = np.zeros((F, T1_ROWS), dtype=np.float32)
    xT_full[:, :N] = x.T
    xT_bf = _bf16(xT_full)
    dinv_pad = np.zeros(T1_ROWS, dtype=np.float32)
    dinv_pad[:N] = dinv
    iota = np.tile(np.arange(P, dtype=np.float32), (P, 1))
    b1r = np.tile(b1[None, :], (P, 1)).astype(np.float32)
    b2r = np.tile(b2[None, :], (P, 1)).astype(np.float32)

    n_tiles = sched["n_tiles"]
    in_maps = []
    for k in range(CORES):
        idx16, dstoff = data[k]
        dv_loc = np.zeros(SHARD_PAD, dtype=np.float32)
        dv_loc[:SHARD] = dinv[SHARD * k : SHARD * (k + 1)]
        dinv_wn = dv_loc.reshape(WINS, P).T.copy()
        dinv_ck = (
            dinv_pad[SHARD_PAD * k : SHARD_PAD * (k + 1)].reshape(WINS, P).T.copy()
        )
        in_maps.append(
            {
                "xT": np.asarray(xT_bf[:, SHARD_PAD * k : SHARD_PAD * (k + 1)]).copy(),
                "w1": np.asarray(_bf16(W1)),
                "w2": np.asarray(_bf16(W2)),
                "b1r": b1r,
                "b2r": b2r,
                "dinv_ch": dinv_ck,
                "dinv_wn": dinv_wn,
                "iota": iota,
                "idx1": _wrap_idx16(idx16),
                "dstoff": dstoff.reshape(n_tiles, P).T.copy(),
            }
        )
    return nc, in_maps


def kernel(x, edge_index, W1, b1, W2, b2):
    from concourse.bass_utils import run_bass_kernel_spmd

    nc, in_maps = _prepare(x, edge_index, W1, b1, W2, b2)
    res = run_bass_kernel_spmd(nc, in_maps, core_ids=list(range(CORES)), trace=False)
    out = np.empty((N, CLS), dtype=np.float32)
    for k in range(CORES):
        out[SHARD * k : SHARD * (k + 1)] = res.results[k]["out"][:SHARD]
    return out


def make_runner(nc, in_maps):
    """Jit once, stage inputs once; returns fn() -> outputs (blocked)."""
    import jax
    import concourse.mybir as mybir
    from jax.sharding import Mesh, PartitionSpec
    from jax.experimental.shard_map import shard_map
    from concourse.bass2jax import (
        _bass_exec_p,
        install_neuronx_cc_hook,
        partition_id_tensor,
    )

    install_neuronx_cc_hook()
    in_names, out_names, out_avals, zero_outs = [], [], [], []
    for alloc in nc.m.functions[0].allocations:
        if not isinstance(alloc, mybir.MemoryLocationSet):
            continue
        name = alloc.memorylocations[0].name
        if alloc.kind == "ExternalInput":
            if nc.partition_id_tensor is None or name != nc.partition_id_tensor.name:
                in_names.append(name)
        elif alloc.kind == "ExternalOutput":
            out_names.append(name)
            shape = tuple(alloc.tensor_shape)
            dtype = mybir.dt.np(alloc.dtype)
            out_avals.append(jax.core.ShapedArray(shape, dtype))
            zero_outs.append(np.zeros(shape, dtype))
    n_params = len(in_names)
    all_in_names = list(in_names) + out_names
    if nc.partition_id_tensor is not None:
        all_in_names.append(nc.partition_id_tensor.name)

    def _body(*args):
        operands = list(args)
        if nc.partition_id_tensor is not None:
            operands.append(partition_id_tensor())
        return tuple(
            _bass_exec_p.bind(
                *operands,
                out_avals=tuple(out_avals),
                in_names=tuple(all_in_names),
                out_names=tuple(out_names),
                lowering_input_output_aliases=(),
                sim_require_finite=True,
                sim_require_nnan=True,
                nc=nc,
            )
        )

    devices = jax.devices()[: len(in_maps)]
    mesh = Mesh(np.asarray(devices), ("core",))
    specs = (PartitionSpec("core"),) * (n_params + len(out_names))
    fn = jax.jit(
        shard_map(
            _body,
            mesh=mesh,
            in_specs=specs,
            out_specs=(PartitionSpec("core"),) * len(out_names),
            check_rep=False,
        ),
        keep_unused=True,
    )
    args = [
        jax.device_put(np.concatenate([np.asarray(m[n]) for m in in_maps], axis=0))
        for n in in_names
    ]
    args += [
        jax.device_put(np.zeros((len(in_maps) * z.shape[0], *z.shape[1:]), z.dtype))
        for z in zero_outs
    ]

    def run():
        out = fn(*args)
        jax.block_until_ready(out)
        return out

    return run
